# revision 10
# baseline (speedup 1.0000x reference)
"""Trainium2 Bass kernel for the Burgers PINN model (nn_BurgersModelWrapper).

Computes, for a 9-layer tanh MLP u(x,t) (widths 2-10-...-10-1):
  u(X_f), u(x0_t0), u(xb_left), u(xb_right), f = u_t + u*u_x - NU*u_xx, u_x, u_xx
using explicit forward-mode jet propagation (streams h, h_x, h_t, h_xx')
instead of autodiff.  Data parallel over 8 NeuronCores; weights replicated.

Layout: packed features-on-partitions: 12 slots x 10 features = 120
partitions; columns = points.  Matmuls use block-diagonal fp16 weights and
write fp16 PSUM (1024 cols per bank); all elementwise work is fp16.
"""

import math
import time

import numpy as np

import concourse.bacc as bacc
import concourse.mybir as mybir
from concourse import tile
from concourse.bass_utils import run_bass_kernel_spmd

# ---------------------------------------------------------------- constants
NU = 0.01 / math.pi
N_F = 1048576
N_0 = 16384
N_B = 16384
N_CORES = 8

S = 12          # point slots per column
F = 10          # hidden width
C = 512         # columns per chunk (= one PSUM bank of fp32)
PTS_PER_CHUNK = S * C          # 12288
NCH_F = 22                     # X_f chunks per core (131072 -> padded 135168)
PTS_F_CORE = N_F // N_CORES          # 131072
PTS_F_PAD = NCH_F * PTS_PER_CHUNK    # 135168
PTS_S_CORE = (N_0 + 2 * N_B) // N_CORES  # 6144 = one chunk
F16 = mybir.dt.float16
F32 = mybir.dt.float32

# wc16 free-dim column offsets
OFF_W1 = 0                   # [24, 120]
OFF_WMID = 120               # 7 mats of [120, 120]: layers 2..8
OFF_W9U = 120 + 7 * 120      # [120, 12]
OFF_W9X = OFF_W9U + 12       # [120, 12]
WC16_COLS = OFF_W9X + 12
# wc32 columns: 0..7 bias vec for layers 1..8; 8 w1x; 9 w1t; 10 w1x^2; 11 b9
WC32_COLS = 12

YF_STREAMS = 5  # u, ux, ut, uxx, f
YF_COLS = NCH_F * C  # 11264


def _build_program():
    nc = bacc.Bacc("TRN2", target_bir_lowering=False, debug=False,
                   num_devices=N_CORES)
    xf = nc.dram_tensor("xf", [24, YF_COLS], F16, kind="ExternalInput").ap()
    xs = nc.dram_tensor("xs", [24, C], F16, kind="ExternalInput").ap()
    wc16 = nc.dram_tensor("wc16", [128, WC16_COLS], F16,
                          kind="ExternalInput").ap()
    wc32 = nc.dram_tensor("wc32", [128, WC32_COLS], F32,
                          kind="ExternalInput").ap()
    yf = nc.dram_tensor("yf", [12, YF_STREAMS * YF_COLS], F16,
                        kind="ExternalOutput").ap()
    ys = nc.dram_tensor("ys", [12, C], F16, kind="ExternalOutput").ap()

    AF = mybir.ActivationFunctionType
    ALU = mybir.AluOpType

    from contextlib import ExitStack
    with tile.TileContext(nc) as tc, ExitStack() as es:
        cpool = es.enter_context(tc.tile_pool(name="consts", bufs=1))
        w16 = cpool.tile([128, WC16_COLS], F16)
        w32 = cpool.tile([128, WC32_COLS], F32)
        nc.sync.dma_start(out=w16[:], in_=wc16[:])
        nc.sync.dma_start(out=w32[:], in_=wc32[:])

        xpool = es.enter_context(tc.tile_pool(name="xin", bufs=3))
        ppool = es.enter_context(tc.tile_pool(name="psum", bufs=2, space="PSUM"))
        spool = es.enter_context(tc.tile_pool(name="streams", bufs=3))
        opool = es.enter_context(tc.tile_pool(name="outs", bufs=3))

        def lhs_mid(l):  # l in 2..8
            return w16[0:120, OFF_WMID + (l - 2) * 120:OFF_WMID + (l - 1) * 120]

        for k in range(NCH_F + 1):
            small = k == NCH_F  # last chunk: boundary/initial pts, fwd only
            xin = xpool.tile([24, C], F16, tag="xin")
            if small:
                nc.sync.dma_start(out=xin[:], in_=xs[:])
            else:
                nc.sync.dma_start(out=xin[:], in_=xf[:, k * C:(k + 1) * C])

            # ---- layer 1 ----
            ps = ppool.tile([128, 4 * C], F32, tag="ps")
            nc.tensor.matmul(out=ps[0:120, 0:C], lhsT=w16[0:24, 0:120],
                             rhs=xin[:], start=True, stop=True)
            h = spool.tile([120, C], F16, tag="h")
            nc.scalar.activation(h[:], ps[0:120, 0:C], AF.Tanh,
                                 bias=w32[0:120, 0:1], scale=1.0)
            if not small:
                sq = spool.tile([120, C], F16, tag="sq")
                d = spool.tile([120, C], F16, tag="d")
                hx = spool.tile([120, C], F16, tag="hx")
                ht = spool.tile([120, C], F16, tag="ht")
                p2 = spool.tile([120, C], F16, tag="p2")
                v = spool.tile([120, C], F16, tag="v")
                nc.vector.tensor_tensor(out=sq[:], in0=h[:], in1=h[:],
                                        op=ALU.mult)
                nc.vector.tensor_scalar(out=d[:], in0=sq[:], scalar1=-1.0,
                                        scalar2=1.0, op0=ALU.mult, op1=ALU.add)
                nc.vector.tensor_scalar(out=hx[:], in0=d[:],
                                        scalar1=w32[0:120, 8:9], scalar2=None,
                                        op0=ALU.mult)
                nc.vector.tensor_scalar(out=ht[:], in0=d[:],
                                        scalar1=w32[0:120, 9:10], scalar2=None,
                                        op0=ALU.mult)
                nc.vector.tensor_tensor(out=p2[:], in0=h[:], in1=d[:],
                                        op=ALU.mult)
                nc.vector.tensor_scalar(out=v[:], in0=p2[:],
                                        scalar1=w32[0:120, 10:11], scalar2=None,
                                        op0=ALU.mult)
                a2 = None

            # ---- layers 2..8 ----
            for l in range(2, 9):
                ps = ppool.tile([128, 4 * C], F32, tag="ps")
                lhsT = lhs_mid(l)
                nc.tensor.matmul(out=ps[0:120, 0:C], lhsT=lhsT, rhs=h[:],
                                 start=True, stop=True)
                if not small:
                    nc.tensor.matmul(out=ps[0:120, C:2 * C], lhsT=lhsT,
                                     rhs=hx[:], start=True, stop=True)
                    nc.tensor.matmul(out=ps[0:120, 2 * C:3 * C], lhsT=lhsT,
                                     rhs=ht[:], start=True, stop=True)
                    nc.tensor.matmul(out=ps[0:120, 3 * C:4 * C], lhsT=lhsT,
                                     rhs=v[:], start=True, stop=(a2 is None))
                    if a2 is not None:
                        nc.tensor.matmul(out=ps[0:120, 3 * C:4 * C], lhsT=lhsT,
                                         rhs=a2[:], start=False, stop=True)
                h = spool.tile([120, C], F16, tag="h")
                nc.scalar.activation(h[:], ps[0:120, 0:C], AF.Tanh,
                                     bias=w32[0:120, l - 1:l], scale=1.0)
                if not small:
                    sq = spool.tile([120, C], F16, tag="sq")
                    d = spool.tile([120, C], F16, tag="d")
                    hx = spool.tile([120, C], F16, tag="hx")
                    ht = spool.tile([120, C], F16, tag="ht")
                    p2 = spool.tile([120, C], F16, tag="p2")
                    v = spool.tile([120, C], F16, tag="v")
                    a2 = spool.tile([120, C], F16, tag="a2")
                    nc.vector.tensor_tensor(out=sq[:], in0=h[:], in1=h[:],
                                            op=ALU.mult)
                    nc.vector.tensor_scalar(out=d[:], in0=sq[:], scalar1=-1.0,
                                            scalar2=1.0, op0=ALU.mult,
                                            op1=ALU.add)
                    nc.vector.tensor_tensor(out=hx[:], in0=d[:],
                                            in1=ps[0:120, C:2 * C],
                                            op=ALU.mult)
                    nc.vector.tensor_tensor(out=ht[:], in0=d[:],
                                            in1=ps[0:120, 2 * C:3 * C],
                                            op=ALU.mult)
                    nc.vector.tensor_tensor(out=v[:], in0=d[:],
                                            in1=ps[0:120, 3 * C:4 * C],
                                            op=ALU.mult)
                    nc.vector.tensor_tensor(out=p2[:], in0=h[:], in1=hx[:],
                                            op=ALU.mult)
                    nc.vector.tensor_tensor(out=a2[:], in0=p2[:],
                                            in1=ps[0:120, C:2 * C],
                                            op=ALU.mult)

            # ---- head (layer 9) ----
            ps = ppool.tile([128, 4 * C], F32, tag="ps")
            w9u = w16[0:120, OFF_W9U:OFF_W9U + 12]
            w9x = w16[0:120, OFF_W9X:OFF_W9X + 12]
            nc.tensor.matmul(out=ps[0:12, 0:C], lhsT=w9u, rhs=h[:],
                             start=True, stop=True)
            if small:
                os_t = opool.tile([12, C], F16, tag="osmall")
                nc.scalar.activation(os_t[:], ps[0:12, 0:C], AF.Copy)
                nc.sync.dma_start(out=ys[:], in_=os_t[:])
            else:
                nc.tensor.matmul(out=ps[0:12, C:2 * C], lhsT=w9u, rhs=hx[:],
                                 start=True, stop=True)
                nc.tensor.matmul(out=ps[0:12, 2 * C:3 * C], lhsT=w9u,
                                 rhs=ht[:], start=True, stop=True)
                nc.tensor.matmul(out=ps[0:12, 3 * C:4 * C], lhsT=w9x,
                                 rhs=v[:], start=True, stop=False)
                nc.tensor.matmul(out=ps[0:12, 3 * C:4 * C], lhsT=w9x,
                                 rhs=a2[:], start=False, stop=True)
                o = opool.tile([12, YF_STREAMS * C], F16, tag="o")
                nc.scalar.activation(o[:, 0:4 * C], ps[0:12, 0:4 * C], AF.Copy)
                ub = opool.tile([12, C], F16, tag="ub")
                fa = opool.tile([12, C], F16, tag="fa")
                fb = opool.tile([12, C], F16, tag="fb")
                fc = opool.tile([12, C], F16, tag="fc")
                # f = ut + (u + b9) * ux - NU * uxx
                nc.vector.tensor_scalar(out=ub[:], in0=o[:, 0:C],
                                        scalar1=w32[0:12, 11:12], scalar2=None,
                                        op0=ALU.add)
                nc.vector.tensor_tensor(out=fa[:], in0=ub[:], in1=o[:, C:2 * C],
                                        op=ALU.mult)
                nc.vector.tensor_tensor(out=fb[:], in0=fa[:],
                                        in1=o[:, 2 * C:3 * C], op=ALU.add)
                nc.vector.tensor_scalar(out=fc[:], in0=o[:, 3 * C:4 * C],
                                        scalar1=-NU, scalar2=None, op0=ALU.mult)
                nc.vector.tensor_tensor(out=o[:, 4 * C:5 * C], in0=fb[:],
                                        in1=fc[:], op=ALU.add)
                dst = yf[:, :].rearrange("p (s n) -> p s n", s=YF_STREAMS)
                nc.sync.dma_start(
                    out=dst[:, :, k * C:(k + 1) * C],
                    in_=o[:].rearrange("p (s n) -> p s n", s=YF_STREAMS))
    nc.compile()
    return nc


# ------------------------------------------------------------- host packing

def _pack_points(pts: np.ndarray, nch: int) -> np.ndarray:
    """[nch*S*C, 2] fp32 -> [24, nch*C] fp16 with row 2s+b."""
    a = pts.reshape(nch, S, C, 2).transpose(1, 3, 0, 2)  # [S, 2, nch, C]
    return np.ascontiguousarray(a.reshape(24, nch * C)).astype(np.float16)


def _unpack_out(y: np.ndarray, nch: int) -> np.ndarray:
    """[12, nch*C] -> [nch*S*C] point order."""
    return np.ascontiguousarray(
        y.reshape(S, nch, C).transpose(1, 0, 2)).reshape(-1)


def _make_consts(inp) -> tuple[np.ndarray, np.ndarray]:
    Ws = [np.asarray(inp[f"W{i}"], np.float32) for i in range(1, 10)]
    bs = [np.asarray(inp[f"b{i}"], np.float32) for i in range(1, 10)]
    wc16 = np.zeros((128, WC16_COLS), np.float16)
    for s in range(S):
        wc16[2 * s:2 * s + 2, OFF_W1 + 10 * s:OFF_W1 + 10 * s + 10] = Ws[0]
    for l in range(2, 9):
        off = OFF_WMID + (l - 2) * 120
        for s in range(S):
            wc16[10 * s:10 * s + 10, off + 10 * s:off + 10 * s + 10] = Ws[l - 1]
    for s in range(S):
        wc16[10 * s:10 * s + 10, OFF_W9U + s] = Ws[8][:, 0]
        wc16[10 * s:10 * s + 10, OFF_W9X + s] = -2.0 * Ws[8][:, 0]
    wc32 = np.zeros((128, WC32_COLS), np.float32)
    for l in range(1, 9):
        wc32[0:120, l - 1] = np.tile(bs[l - 1], S)
    wc32[0:120, 8] = np.tile(Ws[0][0, :], S)
    wc32[0:120, 9] = np.tile(Ws[0][1, :], S)
    wc32[0:120, 10] = np.tile(Ws[0][0, :] ** 2, S)
    wc32[0:12, 11] = bs[8][0]  # b9 for the f-chain
    return wc16, wc32


_CACHE = {}


def _get_program():
    if "nc" not in _CACHE:
        _CACHE["nc"] = _build_program()
    return _CACHE["nc"]


def _make_sharded(nc):
    """Build a cached jitted SPMD executor (mirrors bass2jax.run_bass_via_pjrt)."""
    import jax
    from jax.experimental.shard_map import shard_map
    from jax.sharding import Mesh, PartitionSpec
    from concourse import bass2jax

    bass2jax.install_neuronx_cc_hook()
    partition_name = (nc.partition_id_tensor.name
                      if nc.partition_id_tensor else None)
    in_names, out_names, out_avals, zero_outs = [], [], [], []
    for alloc in nc.m.functions[0].allocations:
        if not isinstance(alloc, mybir.MemoryLocationSet):
            continue
        name = alloc.memorylocations[0].name
        if alloc.kind == "ExternalInput":
            if name != partition_name:
                in_names.append(name)
        elif alloc.kind == "ExternalOutput":
            shape = tuple(alloc.tensor_shape)
            dtype = mybir.dt.np(alloc.dtype)
            out_names.append(name)
            out_avals.append(jax.core.ShapedArray(shape, dtype))
            zero_outs.append(np.zeros(shape, dtype))
    n_params = len(in_names)
    all_in_names = list(in_names) + list(out_names)
    if partition_name is not None:
        all_in_names.append(partition_name)
    donate = tuple(range(n_params, n_params + len(out_names)))

    def _body(*args):
        operands = list(args)
        if partition_name is not None:
            operands.append(bass2jax.partition_id_tensor())
        outs = bass2jax._bass_exec_p.bind(
            *operands,
            out_avals=tuple(out_avals),
            in_names=tuple(all_in_names),
            out_names=tuple(out_names),
            lowering_input_output_aliases=(),
            sim_require_finite=True,
            sim_require_nnan=True,
            nc=nc,
        )
        return tuple(outs)

    devices = jax.devices()[:N_CORES]
    mesh = Mesh(np.asarray(devices), ("core",))
    in_specs = (PartitionSpec("core"),) * (n_params + len(out_names))
    out_specs = (PartitionSpec("core"),) * len(out_names)
    sharded = jax.jit(
        shard_map(_body, mesh=mesh, in_specs=in_specs, out_specs=out_specs,
                  check_rep=False),
        donate_argnums=donate, keep_unused=True)
    return {"fn": sharded, "in_names": in_names, "out_names": out_names,
            "out_avals": out_avals, "zero_outs": zero_outs}


def _get_exec():
    if "exec" not in _CACHE:
        _CACHE["exec"] = _make_sharded(_get_program())
    return _CACHE["exec"]


def _run(in_maps):
    ex = _get_exec()
    concat_in = [np.concatenate([np.asarray(m[name]) for m in in_maps], axis=0)
                 for name in ex["in_names"]]
    concat_zeros = [np.zeros((N_CORES * z.shape[0], *z.shape[1:]), z.dtype)
                    for z in ex["zero_outs"]]
    t0 = time.time()
    out_arrs = ex["fn"](*concat_in, *concat_zeros)
    out_arrs = [np.asarray(o) for o in out_arrs]
    _CACHE["wall_s"] = time.time() - t0
    _CACHE["last_inputs"] = concat_in
    return [
        {name: out_arrs[i].reshape(N_CORES, *ex["out_avals"][i].shape)[c]
         for i, name in enumerate(ex["out_names"])}
        for c in range(N_CORES)
    ]


def bench(n_iter=8):
    """Re-execute the compiled NEFF with device-resident inputs; returns
    per-call wall seconds."""
    import jax
    ex = _get_exec()
    concat_in = _CACHE["last_inputs"]
    dev_in = [jax.device_put(a) for a in concat_in]
    times = []
    for _ in range(n_iter):
        zeros = [np.zeros((N_CORES * z.shape[0], *z.shape[1:]), z.dtype)
                 for z in ex["zero_outs"]]
        dz = [jax.device_put(z) for z in zeros]
        for a in dz:
            a.block_until_ready()
        t0 = time.time()
        out = ex["fn"](*dev_in, *dz)
        for o in out:
            o.block_until_ready()
        times.append(time.time() - t0)
    return times


def kernel(**inputs) -> tuple:
    X_f = np.asarray(inputs["X_f"], np.float32)
    x0 = np.asarray(inputs["x0_t0"], np.float32)
    xbl = np.asarray(inputs["xb_left_tb"], np.float32)
    xbr = np.asarray(inputs["xb_right_tb"], np.float32)
    b9 = float(np.asarray(inputs["b9"])[0])
    wc16, wc32 = _make_consts(inputs)

    nf_core = N_F // N_CORES
    n0_core = N_0 // N_CORES
    nb_core = N_B // N_CORES
    in_maps = []
    for c in range(N_CORES):
        xfc = X_f[c * nf_core:(c + 1) * nf_core]
        pad = np.full((PTS_F_PAD - nf_core, 2), 0.5, np.float32)
        xfp = _pack_points(np.concatenate([xfc, pad]), NCH_F)
        sm = np.concatenate([
            x0[c * n0_core:(c + 1) * n0_core],
            xbl[c * nb_core:(c + 1) * nb_core],
            xbr[c * nb_core:(c + 1) * nb_core],
            np.full((PTS_PER_CHUNK - PTS_S_CORE, 2), 0.5, np.float32)])
        xsp = _pack_points(sm, 1)
        in_maps.append({"xf": xfp, "xs": xsp, "wc16": wc16, "wc32": wc32})

    results = _run(in_maps)

    u_l, f_l, ux_l, uxx_l, u0_l, ubl_l, ubr_l = [], [], [], [], [], [], []
    for c in range(N_CORES):
        yf = np.asarray(results[c]["yf"], np.float32).reshape(
            12, YF_STREAMS, YF_COLS)
        u = _unpack_out(yf[:, 0], NCH_F)[:nf_core] + b9
        ux = _unpack_out(yf[:, 1], NCH_F)[:nf_core]
        uxx = _unpack_out(yf[:, 3], NCH_F)[:nf_core]
        f = _unpack_out(yf[:, 4], NCH_F)[:nf_core]
        ys_ = np.asarray(results[c]["ys"], np.float32)
        usm = _unpack_out(ys_, 1) + b9
        u_l.append(u); f_l.append(f); ux_l.append(ux); uxx_l.append(uxx)
        u0_l.append(usm[0:n0_core])
        ubl_l.append(usm[n0_core:n0_core + nb_core])
        ubr_l.append(usm[n0_core + nb_core:n0_core + 2 * nb_core])

    col = lambda parts: np.concatenate(parts).astype(np.float32)[:, None]
    return (col(u_l), col(u0_l), col(ubl_l), col(ubr_l),
            col(f_l), col(ux_l), col(uxx_l))


# revision 11
# speedup vs baseline: 1.4094x; 1.4094x over previous
"""Trainium2 Bass kernel for the Burgers PINN model (nn_BurgersModelWrapper).

Computes, for a 9-layer tanh MLP u(x,t) (widths 2-10-...-10-1):
  u(X_f), u(x0_t0), u(xb_left), u(xb_right), f = u_t + u*u_x - NU*u_xx, u_x, u_xx
using explicit forward-mode jet propagation (streams h, h_x, h_t, h_xx')
instead of autodiff.  Data parallel over 8 NeuronCores; weights replicated.

Layout: packed features-on-partitions: 12 slots x 10 features = 120
partitions; columns = points.  Matmuls use block-diagonal fp16 weights and
write fp16 PSUM (1024 cols per bank); all elementwise work is fp16.
"""

import math
import time

import numpy as np

import concourse.bacc as bacc
import concourse.mybir as mybir
from concourse import tile
from concourse.bass_utils import run_bass_kernel_spmd

# ---------------------------------------------------------------- constants
NU = 0.01 / math.pi
N_F = 1048576
N_0 = 16384
N_B = 16384
N_CORES = 8

S = 12          # point slots per column
F = 10          # hidden width
C = 512         # columns per chunk (= one PSUM bank of fp32)
PTS_PER_CHUNK = S * C          # 12288
NCH_F = 22                     # X_f chunks per core (131072 -> padded 135168)
PTS_F_CORE = N_F // N_CORES          # 131072
PTS_F_PAD = NCH_F * PTS_PER_CHUNK    # 135168
PTS_S_CORE = (N_0 + 2 * N_B) // N_CORES  # 6144 = one chunk
F16 = mybir.dt.float16
F32 = mybir.dt.float32

# wc16 free-dim column offsets
OFF_W1 = 0                   # [24, 120]
OFF_WMID = 120               # 7 mats of [120, 120]: layers 2..8
OFF_W9U = 120 + 7 * 120      # [120, 12]
OFF_W9X = OFF_W9U + 12       # [120, 12]
WC16_COLS = OFF_W9X + 12
# wc32 columns: 0..7 bias vec for layers 1..8; 8 w1x; 9 w1t; 10 w1x^2; 11 b9
WC32_COLS = 12

YF_STREAMS = 5  # u, ux, ut, uxx, f
YF_COLS = NCH_F * C  # 11264


def _build_program():
    nc = bacc.Bacc("TRN2", target_bir_lowering=False, debug=False,
                   num_devices=N_CORES)
    xf = nc.dram_tensor("xf", [24, YF_COLS], F16, kind="ExternalInput").ap()
    xs = nc.dram_tensor("xs", [24, C], F16, kind="ExternalInput").ap()
    wc16 = nc.dram_tensor("wc16", [128, WC16_COLS], F16,
                          kind="ExternalInput").ap()
    wc32 = nc.dram_tensor("wc32", [128, WC32_COLS], F32,
                          kind="ExternalInput").ap()
    yf = nc.dram_tensor("yf", [12, YF_STREAMS * YF_COLS], F16,
                        kind="ExternalOutput").ap()
    ys = nc.dram_tensor("ys", [12, C], F16, kind="ExternalOutput").ap()

    AF = mybir.ActivationFunctionType
    ALU = mybir.AluOpType

    from contextlib import ExitStack
    with tile.TileContext(nc) as tc, ExitStack() as es:
        cpool = es.enter_context(tc.tile_pool(name="consts", bufs=1))
        w16 = cpool.tile([128, WC16_COLS], F16)
        w32 = cpool.tile([128, WC32_COLS], F32)
        nc.sync.dma_start(out=w16[:], in_=wc16[:])
        nc.sync.dma_start(out=w32[:], in_=wc32[:])

        xpool = es.enter_context(tc.tile_pool(name="xin", bufs=3))
        ppool = es.enter_context(tc.tile_pool(name="psum", bufs=2, space="PSUM"))
        spool = es.enter_context(tc.tile_pool(name="streams", bufs=3))
        opool = es.enter_context(tc.tile_pool(name="outs", bufs=3))

        def lhs_mid(l):  # l in 2..8
            return w16[0:120, OFF_WMID + (l - 2) * 120:OFF_WMID + (l - 1) * 120]

        for k in range(NCH_F + 1):
            small = k == NCH_F  # last chunk: boundary/initial pts, fwd only
            xin = xpool.tile([24, C], F16, tag="xin")
            if small:
                nc.sync.dma_start(out=xin[:], in_=xs[:])
            else:
                nc.sync.dma_start(out=xin[:], in_=xf[:, k * C:(k + 1) * C])

            # ---- layer 1 ----
            ps = ppool.tile([128, 4 * C], F32, tag="ps")
            nc.tensor.matmul(out=ps[0:120, 0:C], lhsT=w16[0:24, 0:120],
                             rhs=xin[:], start=True, stop=True)
            h = spool.tile([120, C], F16, tag="h")
            nc.scalar.activation(h[:], ps[0:120, 0:C], AF.Tanh,
                                 bias=w32[0:120, 0:1], scale=1.0)
            if not small:
                sq = spool.tile([120, C], F32, tag="sq")
                d = spool.tile([120, C], F32, tag="d")
                hx = spool.tile([120, C], F16, tag="hx")
                ht = spool.tile([120, C], F16, tag="ht")
                p2 = spool.tile([120, C], F16, tag="p2")
                v = spool.tile([120, C], F16, tag="v")
                nc.vector.tensor_tensor(out=sq[:], in0=h[:], in1=h[:],
                                        op=ALU.mult)
                nc.vector.tensor_scalar(out=d[:], in0=sq[:], scalar1=-1.0,
                                        scalar2=1.0, op0=ALU.mult, op1=ALU.add)
                nc.vector.tensor_scalar(out=hx[:], in0=d[:],
                                        scalar1=w32[0:120, 8:9], scalar2=None,
                                        op0=ALU.mult)
                nc.vector.tensor_scalar(out=ht[:], in0=d[:],
                                        scalar1=w32[0:120, 9:10], scalar2=None,
                                        op0=ALU.mult)
                nc.vector.tensor_tensor(out=p2[:], in0=h[:], in1=d[:],
                                        op=ALU.mult)
                nc.vector.tensor_scalar(out=v[:], in0=p2[:],
                                        scalar1=w32[0:120, 10:11], scalar2=None,
                                        op0=ALU.mult)
                a2 = None

            # ---- layers 2..8 ----
            for l in range(2, 9):
                ps = ppool.tile([128, 4 * C], F32, tag="ps")
                lhsT = lhs_mid(l)
                nc.tensor.matmul(out=ps[0:120, 0:C], lhsT=lhsT, rhs=h[:],
                                 start=True, stop=True)
                if not small:
                    nc.tensor.matmul(out=ps[0:120, C:2 * C], lhsT=lhsT,
                                     rhs=hx[:], start=True, stop=True)
                    nc.tensor.matmul(out=ps[0:120, 2 * C:3 * C], lhsT=lhsT,
                                     rhs=ht[:], start=True, stop=True)
                    nc.tensor.matmul(out=ps[0:120, 3 * C:4 * C], lhsT=lhsT,
                                     rhs=v[:], start=True, stop=(a2 is None))
                    if a2 is not None:
                        nc.tensor.matmul(out=ps[0:120, 3 * C:4 * C], lhsT=lhsT,
                                         rhs=a2[:], start=False, stop=True)
                h = spool.tile([120, C], F16, tag="h")
                nc.scalar.activation(h[:], ps[0:120, 0:C], AF.Tanh,
                                     bias=w32[0:120, l - 1:l], scale=1.0)
                if not small:
                    sq = spool.tile([120, C], F32, tag="sq")
                    d = spool.tile([120, C], F32, tag="d")
                    hx = spool.tile([120, C], F16, tag="hx")
                    ht = spool.tile([120, C], F16, tag="ht")
                    p2 = spool.tile([120, C], F16, tag="p2")
                    v = spool.tile([120, C], F16, tag="v")
                    a2 = spool.tile([120, C], F16, tag="a2")
                    nc.vector.tensor_tensor(out=sq[:], in0=h[:], in1=h[:],
                                            op=ALU.mult)
                    nc.vector.tensor_scalar(out=d[:], in0=sq[:], scalar1=-1.0,
                                            scalar2=1.0, op0=ALU.mult,
                                            op1=ALU.add)
                    nc.vector.tensor_tensor(out=hx[:], in0=d[:],
                                            in1=ps[0:120, C:2 * C],
                                            op=ALU.mult)
                    nc.vector.tensor_tensor(out=ht[:], in0=d[:],
                                            in1=ps[0:120, 2 * C:3 * C],
                                            op=ALU.mult)
                    nc.vector.tensor_tensor(out=v[:], in0=d[:],
                                            in1=ps[0:120, 3 * C:4 * C],
                                            op=ALU.mult)
                    nc.vector.tensor_tensor(out=p2[:], in0=h[:], in1=hx[:],
                                            op=ALU.mult)
                    nc.vector.tensor_tensor(out=a2[:], in0=p2[:],
                                            in1=ps[0:120, C:2 * C],
                                            op=ALU.mult)

            # ---- head (layer 9) ----
            ps = ppool.tile([128, 4 * C], F32, tag="ps")
            w9u = w16[0:120, OFF_W9U:OFF_W9U + 12]
            w9x = w16[0:120, OFF_W9X:OFF_W9X + 12]
            nc.tensor.matmul(out=ps[0:12, 0:C], lhsT=w9u, rhs=h[:],
                             start=True, stop=True)
            if small:
                os_t = opool.tile([12, C], F16, tag="osmall")
                nc.scalar.activation(os_t[:], ps[0:12, 0:C], AF.Copy)
                nc.sync.dma_start(out=ys[:], in_=os_t[:])
            else:
                nc.tensor.matmul(out=ps[0:12, C:2 * C], lhsT=w9u, rhs=hx[:],
                                 start=True, stop=True)
                nc.tensor.matmul(out=ps[0:12, 2 * C:3 * C], lhsT=w9u,
                                 rhs=ht[:], start=True, stop=True)
                nc.tensor.matmul(out=ps[0:12, 3 * C:4 * C], lhsT=w9x,
                                 rhs=v[:], start=True, stop=False)
                nc.tensor.matmul(out=ps[0:12, 3 * C:4 * C], lhsT=w9x,
                                 rhs=a2[:], start=False, stop=True)
                o = opool.tile([12, YF_STREAMS * C], F16, tag="o")
                nc.scalar.activation(o[:, 0:4 * C], ps[0:12, 0:4 * C], AF.Copy)
                ub = opool.tile([12, C], F16, tag="ub")
                fa = opool.tile([12, C], F16, tag="fa")
                fb = opool.tile([12, C], F16, tag="fb")
                fc = opool.tile([12, C], F16, tag="fc")
                # f = ut + (u + b9) * ux - NU * uxx
                nc.vector.tensor_scalar(out=ub[:], in0=o[:, 0:C],
                                        scalar1=w32[0:12, 11:12], scalar2=None,
                                        op0=ALU.add)
                nc.vector.tensor_tensor(out=fa[:], in0=ub[:], in1=o[:, C:2 * C],
                                        op=ALU.mult)
                nc.vector.tensor_tensor(out=fb[:], in0=fa[:],
                                        in1=o[:, 2 * C:3 * C], op=ALU.add)
                nc.vector.tensor_scalar(out=fc[:], in0=o[:, 3 * C:4 * C],
                                        scalar1=-NU, scalar2=None, op0=ALU.mult)
                nc.vector.tensor_tensor(out=o[:, 4 * C:5 * C], in0=fb[:],
                                        in1=fc[:], op=ALU.add)
                dst = yf[:, :].rearrange("p (s n) -> p s n", s=YF_STREAMS)
                nc.sync.dma_start(
                    out=dst[:, :, k * C:(k + 1) * C],
                    in_=o[:].rearrange("p (s n) -> p s n", s=YF_STREAMS))
    nc.compile()
    return nc


# ------------------------------------------------------------- host packing

def _pack_points(pts: np.ndarray, nch: int) -> np.ndarray:
    """[nch*S*C, 2] fp32 -> [24, nch*C] fp16 with row 2s+b."""
    a = pts.reshape(nch, S, C, 2).transpose(1, 3, 0, 2)  # [S, 2, nch, C]
    return np.ascontiguousarray(a.reshape(24, nch * C)).astype(np.float16)


def _unpack_out(y: np.ndarray, nch: int) -> np.ndarray:
    """[12, nch*C] -> [nch*S*C] point order."""
    return np.ascontiguousarray(
        y.reshape(S, nch, C).transpose(1, 0, 2)).reshape(-1)


def _make_consts(inp) -> tuple[np.ndarray, np.ndarray]:
    Ws = [np.asarray(inp[f"W{i}"], np.float32) for i in range(1, 10)]
    bs = [np.asarray(inp[f"b{i}"], np.float32) for i in range(1, 10)]
    wc16 = np.zeros((128, WC16_COLS), np.float16)
    for s in range(S):
        wc16[2 * s:2 * s + 2, OFF_W1 + 10 * s:OFF_W1 + 10 * s + 10] = Ws[0]
    for l in range(2, 9):
        off = OFF_WMID + (l - 2) * 120
        for s in range(S):
            wc16[10 * s:10 * s + 10, off + 10 * s:off + 10 * s + 10] = Ws[l - 1]
    for s in range(S):
        wc16[10 * s:10 * s + 10, OFF_W9U + s] = Ws[8][:, 0]
        wc16[10 * s:10 * s + 10, OFF_W9X + s] = -2.0 * Ws[8][:, 0]
    wc32 = np.zeros((128, WC32_COLS), np.float32)
    for l in range(1, 9):
        wc32[0:120, l - 1] = np.tile(bs[l - 1], S)
    wc32[0:120, 8] = np.tile(Ws[0][0, :], S)
    wc32[0:120, 9] = np.tile(Ws[0][1, :], S)
    wc32[0:120, 10] = np.tile(Ws[0][0, :] ** 2, S)
    wc32[0:12, 11] = bs[8][0]  # b9 for the f-chain
    return wc16, wc32


_CACHE = {}


def _get_program():
    if "nc" not in _CACHE:
        _CACHE["nc"] = _build_program()
    return _CACHE["nc"]


def _make_sharded(nc):
    """Build a cached jitted SPMD executor (mirrors bass2jax.run_bass_via_pjrt)."""
    import jax
    from jax.experimental.shard_map import shard_map
    from jax.sharding import Mesh, PartitionSpec
    from concourse import bass2jax

    bass2jax.install_neuronx_cc_hook()
    partition_name = (nc.partition_id_tensor.name
                      if nc.partition_id_tensor else None)
    in_names, out_names, out_avals, zero_outs = [], [], [], []
    for alloc in nc.m.functions[0].allocations:
        if not isinstance(alloc, mybir.MemoryLocationSet):
            continue
        name = alloc.memorylocations[0].name
        if alloc.kind == "ExternalInput":
            if name != partition_name:
                in_names.append(name)
        elif alloc.kind == "ExternalOutput":
            shape = tuple(alloc.tensor_shape)
            dtype = mybir.dt.np(alloc.dtype)
            out_names.append(name)
            out_avals.append(jax.core.ShapedArray(shape, dtype))
            zero_outs.append(np.zeros(shape, dtype))
    n_params = len(in_names)
    all_in_names = list(in_names) + list(out_names)
    if partition_name is not None:
        all_in_names.append(partition_name)
    donate = tuple(range(n_params, n_params + len(out_names)))

    def _body(*args):
        operands = list(args)
        if partition_name is not None:
            operands.append(bass2jax.partition_id_tensor())
        outs = bass2jax._bass_exec_p.bind(
            *operands,
            out_avals=tuple(out_avals),
            in_names=tuple(all_in_names),
            out_names=tuple(out_names),
            lowering_input_output_aliases=(),
            sim_require_finite=True,
            sim_require_nnan=True,
            nc=nc,
        )
        return tuple(outs)

    devices = jax.devices()[:N_CORES]
    mesh = Mesh(np.asarray(devices), ("core",))
    in_specs = (PartitionSpec("core"),) * (n_params + len(out_names))
    out_specs = (PartitionSpec("core"),) * len(out_names)
    sharded = jax.jit(
        shard_map(_body, mesh=mesh, in_specs=in_specs, out_specs=out_specs,
                  check_rep=False),
        donate_argnums=donate, keep_unused=True)
    return {"fn": sharded, "in_names": in_names, "out_names": out_names,
            "out_avals": out_avals, "zero_outs": zero_outs}


def _get_exec():
    if "exec" not in _CACHE:
        _CACHE["exec"] = _make_sharded(_get_program())
    return _CACHE["exec"]


def _run(in_maps):
    ex = _get_exec()
    concat_in = [np.concatenate([np.asarray(m[name]) for m in in_maps], axis=0)
                 for name in ex["in_names"]]
    concat_zeros = [np.zeros((N_CORES * z.shape[0], *z.shape[1:]), z.dtype)
                    for z in ex["zero_outs"]]
    t0 = time.time()
    out_arrs = ex["fn"](*concat_in, *concat_zeros)
    out_arrs = [np.asarray(o) for o in out_arrs]
    _CACHE["wall_s"] = time.time() - t0
    _CACHE["last_inputs"] = concat_in
    return [
        {name: out_arrs[i].reshape(N_CORES, *ex["out_avals"][i].shape)[c]
         for i, name in enumerate(ex["out_names"])}
        for c in range(N_CORES)
    ]


def bench(n_iter=8):
    """Re-execute the compiled NEFF with device-resident inputs; returns
    per-call wall seconds."""
    import jax
    ex = _get_exec()
    concat_in = _CACHE["last_inputs"]
    dev_in = [jax.device_put(a) for a in concat_in]
    times = []
    for _ in range(n_iter):
        zeros = [np.zeros((N_CORES * z.shape[0], *z.shape[1:]), z.dtype)
                 for z in ex["zero_outs"]]
        dz = [jax.device_put(z) for z in zeros]
        for a in dz:
            a.block_until_ready()
        t0 = time.time()
        out = ex["fn"](*dev_in, *dz)
        for o in out:
            o.block_until_ready()
        times.append(time.time() - t0)
    return times


def kernel(**inputs) -> tuple:
    X_f = np.asarray(inputs["X_f"], np.float32)
    x0 = np.asarray(inputs["x0_t0"], np.float32)
    xbl = np.asarray(inputs["xb_left_tb"], np.float32)
    xbr = np.asarray(inputs["xb_right_tb"], np.float32)
    b9 = float(np.asarray(inputs["b9"])[0])
    wc16, wc32 = _make_consts(inputs)

    nf_core = N_F // N_CORES
    n0_core = N_0 // N_CORES
    nb_core = N_B // N_CORES
    in_maps = []
    for c in range(N_CORES):
        xfc = X_f[c * nf_core:(c + 1) * nf_core]
        pad = np.full((PTS_F_PAD - nf_core, 2), 0.5, np.float32)
        xfp = _pack_points(np.concatenate([xfc, pad]), NCH_F)
        sm = np.concatenate([
            x0[c * n0_core:(c + 1) * n0_core],
            xbl[c * nb_core:(c + 1) * nb_core],
            xbr[c * nb_core:(c + 1) * nb_core],
            np.full((PTS_PER_CHUNK - PTS_S_CORE, 2), 0.5, np.float32)])
        xsp = _pack_points(sm, 1)
        in_maps.append({"xf": xfp, "xs": xsp, "wc16": wc16, "wc32": wc32})

    results = _run(in_maps)

    u_l, f_l, ux_l, uxx_l, u0_l, ubl_l, ubr_l = [], [], [], [], [], [], []
    for c in range(N_CORES):
        yf = np.asarray(results[c]["yf"], np.float32).reshape(
            12, YF_STREAMS, YF_COLS)
        u = _unpack_out(yf[:, 0], NCH_F)[:nf_core] + b9
        ux = _unpack_out(yf[:, 1], NCH_F)[:nf_core]
        uxx = _unpack_out(yf[:, 3], NCH_F)[:nf_core]
        f = _unpack_out(yf[:, 4], NCH_F)[:nf_core]
        ys_ = np.asarray(results[c]["ys"], np.float32)
        usm = _unpack_out(ys_, 1) + b9
        u_l.append(u); f_l.append(f); ux_l.append(ux); uxx_l.append(uxx)
        u0_l.append(usm[0:n0_core])
        ubl_l.append(usm[n0_core:n0_core + nb_core])
        ubr_l.append(usm[n0_core + nb_core:n0_core + 2 * nb_core])

    col = lambda parts: np.concatenate(parts).astype(np.float32)[:, None]
    return (col(u_l), col(u0_l), col(ubl_l), col(ubr_l),
            col(f_l), col(ux_l), col(uxx_l))
